# revision 9
# baseline (speedup 1.0000x reference)
"""Trainium2 Bass kernel for nn_PsiModel2d_83202106458323 — v3.

Computes, for N=4194304 particles with F in R^{N x 2 x 2}:
    C = F^T F; tr = trace(C); delta = sqrt(max(tr^2 - 4 det C, 1e-8))
    sigma = 0.5 (tr +- delta);  out = MLP_{2-16-16-16-1}(sigma1, sigma2)

Key reformulation vs the earlier kernel: the first layer only needs TWO
features per particle,
    feat0 = 2*tr = p + m,  feat1 = delta = sqrt(p*m + eps)
      (p = (a+d)^2 + (b-c)^2, m = (a-d)^2 + (b+c)^2)
since sigma1*W1[0] + sigma2*W1[1] = tr*(W1[0]+W1[1])/2 + delta*(W1[0]-W1[1])/2.
That halves the feature-transpose volume and removes the pad/memset.

Per core (data parallel over 8 cores), per span of 128*T particles:
  - planar elementwise preamble runs ENTIRELY on GPSIMD (u-planes, squares
    as u*u, the fused P|M pair op, pm, 2tr) — GPSIMD cannot touch PSUM
    (walrus ISA rule), so it owns all the SBUF-side prep while DVE/ACT do
    the PSUM work; sqrt(pm+eps) on ACT
  - one DVE 32x32 block transpose TD -> fp32 Rf, then an all-SBUF DVE copy
    to fp32r R [128, 2T] (StreamTranspose cannot emit fp32r; the copy takes
    the 2x_2p DVE perf mode). Column 32b+j holds 64 particles: 4 strips i x
    16 t_sub, 2 feats each.
  - L1: 8 stationaries (strip i = g//2, t_sub half g%2) x full-width matmuls
    -> H1 g-major [128, 8*2T]; L2/L3: blockdiag(8xW) matmuls per g-block
  - PSUM->SBUF evacuation fused with bias+relu, alternating strictly between
    ACT activation and DVE tensor_scalar (the only PSUM-capable engines)
  - L4: 16 sparse stationaries (g x b-parity) accumulate into a DENSE
    [128, T] psum laid out so psum4[32i + r', 32c + j] = out(32i+j, 32c+r');
    b4 is added by one extra ones-row K=1 matmul; a single [128, T] DVE
    block transpose evacuates psum4 straight into the contiguous output tile
  - contiguous output DMA (1KB descriptors)

Stationaries live in one [128, NW] input declared fp32r in DRAM (same bits
as fp32) so no on-chip conversion copy is needed; biases ride a tiny fp32
side input. The next span's preamble is emitted between a span's L1 and L2
so the tile scheduler can overlap it into PE-heavy stretches.
"""
import sys

sys.path.insert(0, "/opt/trn_rl_repo")
import numpy as np
import concourse.bass as bass
import concourse.tile as tile
from concourse import mybir
from concourse.vector_clock import ScopedClock

FP = mybir.dt.float32
FPR = mybir.dt.float32r
BF = mybir.dt.bfloat16
NCORES = 8
T_DEF = 256        # particles per partition per span
NSPANS_DEF = 16    # spans per core; per-core N = 128 * T * nspans

# wpack column map (see pack_weights); wpack is fp32r end-to-end (same bits
# as fp32 — declared fp32r in DRAM so no on-chip conversion copy is needed)
C_L1 = 0           # 8 x 128
C_B4ROW = 1024     # [1, 128] b4 row (partition 0)
C_ONES = 1152      # [1, T] ones row (partition 0) — first DMA ends at C_W2
C_W2 = lambda T: 1152 + T   # 128
C_W3 = lambda T: 1280 + T   # 128
C_L4 = lambda T: 1408 + T   # 16 x 128
NW_OF = lambda T: 3456 + T

# evacuation engine schedule: 12 evacs per span (4 per hidden layer),
# 'A' = ACT activation, 'D' = DVE tensor_scalar, 'P' = GPSIMD tensor_scalar
EVAC_SCHED = "ADAD AADA ADAD".replace(" ", "")


class TC(tile.TileContext):
    """TileContext whose final drain splits sem waits across NOPs (the nix
    walrus rejects instructions carrying more than one sync wait)."""

    def _drain_and_barrier(self, tick_clock, wait_clock):
        nc = self.nc
        collector = nc.sync.nop(nofuse=True)
        wait_clock.add_sem_waits(
            collector.ins, ScopedClock({None: tick_clock.global_clock})
        )
        si = collector.ins.sync_info
        waits = list(si.on_wait) if si is not None else []
        if si is not None and len(waits) > 1:
            si.on_wait = waits[:1]
            for w in waits[1:]:
                extra = nc.sync.nop(nofuse=True)
                extra.ins.sync_info = mybir.SyncInfo(on_wait=[w], on_update=[])
        nc.sync.drain()
        nc.all_engine_barrier()
        popped = nc._tile_sem_poison_stack.pop()
        assert popped is self._sem_poison
        nc.clear_and_free_semaphores(list(self.sems.allocated().values()))
        nc.all_engine_barrier()


def split_sync_waits(nc, max_waits=1):
    """Move excess per-instruction sync waits onto NOPs inserted just before
    the offending instruction on the same engine (same-engine program order
    preserves semantics)."""
    for fn in nc.m.functions:
        for blk in fn.blocks:
            i = 0
            while i < len(blk.instructions):
                inst = blk.instructions[i]
                si = getattr(inst, "sync_info", None)
                if si is not None and len(si.on_wait) > max_waits:
                    waits = list(si.on_wait)
                    si.on_wait = waits[:max_waits]
                    extra = waits[max_waits:]
                    ninserted = 0
                    while extra:
                        chunk, extra = extra[:max_waits], extra[max_waits:]
                        nop = mybir.InstNoOp(
                            name=nc.get_next_instruction_name(), ins=[], outs=[]
                        )
                        nop.engine = inst.engine
                        nop.sync_info = mybir.SyncInfo(on_wait=chunk, on_update=[])
                        nc.register_instruction(nop)
                        blk.instructions.insert(i, nop)
                        ninserted += 1
                    i += ninserted
                i += 1


def pack_weights(W1, b1, W2, b2, W3, b3, W4, b4, T=T_DEF):
    """Host-side stationary layouts -> ([128, NW] fp32 wpack, [128, 4] bvec)."""
    NW = NW_OF(T)
    wt = ((W1[0] + W1[1]) / 4.0).astype(np.float32)   # applied to 2*tr
    wd = ((W1[0] - W1[1]) / 2.0).astype(np.float32)   # applied to delta
    wp = np.zeros((128, NW), np.float32)
    # L1: S1[g][32*(g//2) + 2*(8*(g%2)+s) + f, 16*s + u] = wt/wd
    for g in range(8):
        i, h = g // 2, g % 2
        blk = wp[:, C_L1 + 128 * g:C_L1 + 128 * g + 128]
        for s in range(8):
            q = 32 * i + 2 * (8 * h + s)
            blk[q + 0, 16 * s:16 * s + 16] = wt
            blk[q + 1, 16 * s:16 * s + 16] = wd
    wp[0, C_B4ROW:C_B4ROW + 128] = b4[0]
    wp[0, C_ONES:C_ONES + T] = 1.0
    # W2/W3 blockdiag
    cw2, cw3, cl4 = C_W2(T), C_W3(T), C_L4(T)
    for s in range(8):
        wp[16 * s:16 * s + 16, cw2 + 16 * s:cw2 + 16 * s + 16] = W2
        wp[16 * s:16 * s + 16, cw3 + 16 * s:cw3 + 16 * s + 16] = W3
    # L4: S4[g,P][16*s + u, 32*i + 16*P + 8*h + s] = W4[u]
    for g in range(8):
        i, h = g // 2, g % 2
        for P in range(2):
            blk = wp[:, cl4 + 128 * (2 * g + P):cl4 + 128 * (2 * g + P) + 128]
            for s in range(8):
                blk[16 * s:16 * s + 16, 32 * i + 16 * P + 8 * h + s] = W4[:, 0]
    import ml_dtypes
    w1b = wp[:, C_L1:C_L1 + 1024].astype(ml_dtypes.bfloat16)
    bv = np.zeros((128, 4), np.float32)
    bv[:, 0] = np.tile(b1, 8)
    bv[:, 1] = np.tile(b2, 8)
    bv[:, 2] = np.tile(b3, 8)
    bv[:, 3] = 1e-8                # EPS bias for the Sqrt activation
    return wp, bv, w1b


def build_program(T=T_DEF, nspans=NSPANS_DEF, num_devices=NCORES,
                  evac_sched=EVAC_SCHED, psp_bufs=3, ps4_bufs=2,
                  evac_split=False, preamble_assign="PPPPPPPPPD",
                  feat_bf16=False, ramp_chunks=True, span_alt=False, evac_sched2=None,
                  l4_deprio=25, tr_prio=0):
    """Build the per-core Bass program. Per-core N = 128*T*nspans."""
    W2T = 2 * T        # transposed tile width
    NB = W2T // 32     # 32-col blocks in R (b index range)
    assert T % 32 == 0 and NB % 2 == 0
    NW = NW_OF(T)
    CW2, CW3, CL4 = C_W2(T), C_W3(T), C_L4(T)
    FDT = BF if feat_bf16 else FP

    nc = bass.Bass("TRN2", target_bir_lowering=False, debug=False,
                   num_devices=num_devices)
    f_in = nc.dram_tensor("f", [nspans, 128, 4 * T], FP, kind="ExternalInput").ap()
    wp_in = nc.dram_tensor("wpack", [128, NW], FPR, kind="ExternalInput").ap()
    bv_in = nc.dram_tensor("bvec", [128, 4], FP, kind="ExternalInput").ap()
    w1_in = nc.dram_tensor("w1b", [128, 1024], BF, kind="ExternalInput").ap()
    out_d = nc.dram_tensor("out", [nspans, 128, T], FP,
                           kind="ExternalOutput").ap()

    add, mx, sub, mult = (mybir.AluOpType.add, mybir.AluOpType.max,
                          mybir.AluOpType.subtract, mybir.AluOpType.mult)
    Relu = mybir.ActivationFunctionType.Relu
    Sqrt = mybir.ActivationFunctionType.Sqrt
    Square = mybir.ActivationFunctionType.Square

    with TC(nc) as tc:
        with (
            tc.tile_pool(name="const", bufs=1) as constp,
            tc.tile_pool(name="io", bufs=3) as iop,
            tc.tile_pool(name="mid", bufs=2) as midp,
            tc.tile_pool(name="r", bufs=2) as rp,
            tc.tile_pool(name="acts", bufs=2) as actp,
            tc.tile_pool(name="ps", bufs=psp_bufs, space="PSUM") as psp,
            tc.tile_pool(name="ps4", bufs=ps4_bufs, space="PSUM") as ps4p,
        ):
            # fp32r end-to-end: no on-chip conversion copy. The L1
            # stationaries (+ biases) load first; W2/W3 and the big L4 block
            # are emitted AFTER span 0's input DMA so they queue behind it on
            # the DMA path (they are not needed until L2/L4 of span 0, but
            # issued early they delay X(0) by several microseconds).
            wsr = constp.tile([128, NW], FPR)
            bvt = constp.tile([128, 4], FP)
            nc.sync.dma_start(bvt[:, :], bv_in)
            w1sb = constp.tile([128, 1024], BF)

            def load_l1_weights():
                nc.sync.dma_start(wsr[:, 0:CW2], wp_in[:, 0:CW2])
                if feat_bf16:
                    nc.sync.dma_start(w1sb[:, :], w1_in)

            def load_late_weights():
                nc.sync.dma_start(wsr[:, CW2:CL4], wp_in[:, CW2:CL4])
                nc.sync.dma_start(wsr[:, CL4:NW], wp_in[:, CL4:NW])
            b1v = bvt[:, 0:1]
            b2v = bvt[:, 1:2]
            b3v = bvt[:, 2:3]
            epsv = bvt[:, 3:4]
            b4row = wsr[0:1, C_B4ROW:C_B4ROW + 128]
            ones = wsr[0:1, C_ONES:C_ONES + T]

            warm = constp.tile([128, 2], FP)
            nc.gpsimd.memset(warm[:, :], 0.0)
            nc.scalar.activation(warm[:, 1:2], warm[:, 0:1], Square)

            def evac(kind, dst, src, bias):
                if kind == "D":
                    nc.vector.tensor_scalar(dst, src, bias, 0.0, add, mx)
                elif kind == "A":
                    nc.scalar.activation(dst, src, Relu, bias=bias)
                else:
                    nc.gpsimd.tensor_scalar(dst, src, bias, 0.0, add, mx)

            def stage_A(sp):
                """DMA + elementwise preamble + feature transpose -> R."""
                X = iop.tile([128, 4 * T], FP, tag="X")
                nc.sync.dma_start(X[:, :], f_in[sp])
                X4 = X.rearrange("p (t k) -> p t k", k=4)

                eng = {"D": nc.vector, "P": nc.gpsimd}
                pa = preamble_assign  # 7 chars: u0 u1 u2 u3 P M PM
                U = midp.tile([128, 4 * T], FP, tag="U")  # planar u0..u3
                eng[pa[0]].tensor_tensor(U[:, 0:T], X4[:, :, 0], X4[:, :, 3], add)
                eng[pa[1]].tensor_tensor(U[:, T:2 * T], X4[:, :, 1], X4[:, :, 2], sub)
                eng[pa[2]].tensor_tensor(U[:, 2 * T:3 * T], X4[:, :, 0], X4[:, :, 3], sub)
                eng[pa[3]].tensor_tensor(U[:, 3 * T:4 * T], X4[:, :, 1], X4[:, :, 2], add)

                V = midp.tile([128, 4 * T], FP, tag="V")
                if len(pa) > 7 and pa[7] in "DP":
                    eng[pa[7]].tensor_tensor(V[:, :], U[:, :], U[:, :], mult)
                else:
                    nc.scalar.activation(V[:, :], U[:, :], Square)

                # P = u0^2 + u1^2 and M = u2^2 + u3^2 fused into one 2T-wide
                # op: in0 walks (V0, V2), in1 walks (V1, V3) via a [2, 2T]
                # view of the planar V tile.
                PMp = midp.tile([128, 2 * T], FP, tag="PMp")
                PM = midp.tile([128, T], FP, tag="PM")
                Vg = V.rearrange("p (g t) -> p g t", g=2)
                eng[pa[4]].tensor_tensor(
                    PMp.rearrange("p (g t) -> p g t", g=2)[:, :, :],
                    Vg[:, :, 0:T], Vg[:, :, T:2 * T], add)
                P = PMp[:, 0:T]
                M = PMp[:, T:2 * T]
                eng[pa[6]].tensor_tensor(PM[:, :], P, M, mult)

                TD = midp.tile([128, W2T], FDT, tag="TD")  # (2tr, delta) pairs
                TD2 = TD.rearrange("p (t k) -> p t k", k=2)
                td0e = eng[pa[8]] if len(pa) > 8 and pa[8] in "DP" else nc.vector
                td0e.tensor_tensor(TD2[:, :, 0], P, M, add)
                nc.scalar.activation(TD2[:, :, 1], PM[:, :], Sqrt, bias=epsv)

                if feat_bf16:
                    # bf16 features: StreamTranspose supports bf16, so R comes
                    # straight out of the transpose (no conversion copy); the
                    # L1 matmuls run bf16 stationary x bf16 moving at the same
                    # 1 column/cycle as fp32r.
                    R = rp.tile([128, W2T], FDT, tag="R")
                    nc.vector.transpose(R[:, :], TD[:, :])
                    return R
                # StreamTranspose cannot emit fp32r (walrus ISA check); do the
                # transpose in fp32 and convert with an all-SBUF copy (on DVE
                # this qualifies for the 2x_2p perf mode). Hoisted in
                # scheduler priority: they gate the next span's L1 matmuls
                # and otherwise queue behind this span's DVE evacuations.
                Rf = midp.tile([128, W2T], FP, tag="Rf")
                R = rp.tile([128, W2T], FPR, tag="R")
                with tc.high_priority(offset=tr_prio):
                    nc.vector.transpose(Rf[:, :], TD[:, :])
                    rce = eng[pa[9]] if len(pa) > 9 and pa[9] in "DP" else nc.vector
                    rce.tensor_copy(R[:, :], Rf[:, :])
                return R

            def layer(sp, ev, lname, Hdst, bias, lhs_of):
                # 4 psum tiles of 2 g-blocks each
                for gg in range(4):
                    ps = psp.tile([128, 2 * W2T], FP, tag="ps",
                                  name=f"{lname}_{sp}_{gg}")
                    for g2 in range(2):
                        g = 2 * gg + g2
                        lhs, rhs = lhs_of(g)
                        nc.tensor.matmul(
                            ps[:, W2T * g2:W2T * g2 + W2T], lhs, rhs,
                            start=True, stop=True)
                    ev_k = next(ev)
                    if ev_k == "S":
                        half = W2T
                        nc.vector.tensor_scalar(
                            Hdst[:, 2 * W2T * gg:2 * W2T * gg + half],
                            ps[:, 0:half], bias, 0.0, add, mx)
                        nc.scalar.activation(
                            Hdst[:, 2 * W2T * gg + half:2 * W2T * (gg + 1)],
                            ps[:, half:2 * half], Relu, bias=bias)
                    elif evac_split:
                        # two half-width evacs on different engines: halves
                        # the psum-tile turnaround latency
                        for g2 in range(2):
                            evac(ev_k if g2 == 0 else next(ev),
                                 Hdst[:, W2T * (2 * gg + g2):
                                      W2T * (2 * gg + g2 + 1)],
                                 ps[:, W2T * g2:W2T * g2 + W2T], bias)
                    else:
                        evac(ev_k, Hdst[:, 2 * W2T * gg:2 * W2T * (gg + 1)],
                             ps[:, :], bias)

            def stage_B1(sp, R, ev):
                """L1 matmuls + evacs -> H1."""
                H1 = actp.tile([128, 8 * W2T], FPR, tag="H1")
                l1s = w1sb if feat_bf16 else wsr
                layer(sp, ev, "l1", H1, b1v,
                      lambda g: (l1s[:, C_L1 + 128 * g:C_L1 + 128 * g + 128],
                                 R[:, :]))
                return H1

            def stage_B2(sp, H1, ev):
                """L2..L4 + bias + output transpose + store."""
                H2 = actp.tile([128, 8 * W2T], FPR, tag="H2")
                H3 = actp.tile([128, 8 * W2T], FPR, tag="H3")
                layer(sp, ev, "l2", H2, b2v,
                      lambda g: (wsr[:, CW2:CW2 + 128],
                                 H1[:, W2T * g:W2T * g + W2T]))
                layer(sp, ev, "l3", H3, b3v,
                      lambda g: (wsr[:, CW3:CW3 + 128],
                                 H2[:, W2T * g:W2T * g + W2T]))

                # ---- L4: dense psum accumulation + ones-row bias ----
                # Deprioritized for the tile scheduler: at span boundaries
                # the PE otherwise interleaves these 16 accumulation matmuls
                # ahead of the next span's L1 matmuls, starving the evac
                # engines of L1 psums (periodic ~1.6us ACT stalls).
                ps4 = ps4p.tile([128, T], FP, tag="ps4")
                H3r = H3.rearrange("p (g c P j) -> p g c P j", g=8, c=NB // 2, P=2)
                with tc.high_priority(offset=(-l4_deprio if sp < nspans - 1 else 15)):
                    k = 0
                    for g in range(8):
                        for Pb in range(2):
                            nc.tensor.matmul(
                                ps4[:, :],
                                wsr[:, CL4 + 128 * (2 * g + Pb):
                                    CL4 + 128 * (2 * g + Pb) + 128],
                                H3r[:, g, :, Pb, :],
                                start=(k == 0), stop=False)
                            k += 1
                    nc.tensor.matmul(ps4[:, :], b4row, ones,
                                     start=False, stop=True)

                Y = iop.tile([128, T], FP, tag="Y")
                nc.vector.transpose(Y[:, :], ps4[:, :])
                nc.sync.dma_start(out_d[sp], Y[:, :])

            def stage_A_quartered(sp):
                """Ramp span: same preamble emitted as 4 column-quarters so
                the first R columns are ready much sooner (all ops are plain
                column slices; T/4 keeps the 32-wide transpose blocks
                aligned)."""
                TQ = T // 4
                X = iop.tile([128, 4 * T], FP, tag="X")
                nc.sync.dma_start(X[:, :], f_in[sp])
                X4 = X.rearrange("p (t k) -> p t k", k=4)
                U = midp.tile([128, 4 * T], FP, tag="U")
                V = midp.tile([128, 4 * T], FP, tag="V")
                U4 = U.rearrange("p (g t) -> p g t", g=4)
                V4 = V.rearrange("p (g t) -> p g t", g=4)
                Vg = V.rearrange("p (g t) -> p g t", g=2)
                PMp = midp.tile([128, 2 * T], FP, tag="PMp")
                PMp2 = PMp.rearrange("p (g t) -> p g t", g=2)
                PM = midp.tile([128, T], FP, tag="PM")
                TD = midp.tile([128, W2T], FDT, tag="TD")
                TD2 = TD.rearrange("p (t k) -> p t k", k=2)
                Rf = midp.tile([128, W2T], FP, tag="Rf")
                R = rp.tile([128, W2T], FPR, tag="R")
                for q in range(4):
                    ts = slice(TQ * q, TQ * (q + 1))
                    tsM = slice(T + TQ * q, T + TQ * (q + 1))
                    nc.gpsimd.tensor_tensor(U4[:, 0, ts], X4[:, ts, 0], X4[:, ts, 3], add)
                    nc.gpsimd.tensor_tensor(U4[:, 1, ts], X4[:, ts, 1], X4[:, ts, 2], sub)
                    nc.gpsimd.tensor_tensor(U4[:, 2, ts], X4[:, ts, 0], X4[:, ts, 3], sub)
                    nc.gpsimd.tensor_tensor(U4[:, 3, ts], X4[:, ts, 1], X4[:, ts, 2], add)
                    nc.scalar.activation(V4[:, :, ts], U4[:, :, ts], Square)
                    nc.gpsimd.tensor_tensor(PMp2[:, :, ts], Vg[:, :, ts],
                                            Vg[:, :, tsM], add)
                    nc.gpsimd.tensor_tensor(PM[:, ts], PMp[:, ts], PMp[:, tsM], mult)
                    nc.gpsimd.tensor_tensor(TD2[:, ts, 0], PMp[:, ts], PMp[:, tsM], add)
                    nc.scalar.activation(TD2[:, ts, 1], PM[:, ts], Sqrt, bias=epsv)
                    cs = slice(2 * TQ * q, 2 * TQ * (q + 1))
                    nc.vector.transpose(Rf[:, cs], TD[:, cs])
                    nc.vector.tensor_copy(R[:, cs], Rf[:, cs])
                return R

            def layer_half(sp, ev, lname, Hdst, bias, lhs_of):
                """Half-width layer for the ramp span: 8 one-bank psum tiles
                per layer instead of 4 two-bank ones (they share the "ps"
                tag's arena slots)."""
                HH = W2T // 2
                Hr = Hdst.rearrange("p (g n) -> p g n", g=8)
                for hh in range(2):
                    for gg in range(4):
                        ps = psp.tile([128, W2T], FP, tag="ps",
                                      name=f"{lname}h_{sp}_{gg}_{hh}")
                        for g2 in range(2):
                            g = 2 * gg + g2
                            lhs, rhs = lhs_of(g, hh)
                            nc.tensor.matmul(
                                ps[:, HH * g2:HH * g2 + HH], lhs, rhs,
                                start=True, stop=True)
                        evac(next(ev),
                             Hr[:, 2 * gg:2 * gg + 2, HH * hh:HH * hh + HH],
                             ps.rearrange("p (g n) -> p g n", g=2)[:, :, :],
                             bias)

            def stage_B1_half(sp, R, ev):
                HH = W2T // 2
                H1 = actp.tile([128, 8 * W2T], FPR, tag="H1")
                l1s = w1sb if feat_bf16 else wsr
                layer_half(sp, ev, "l1", H1, b1v,
                           lambda g, hh: (
                               l1s[:, C_L1 + 128 * g:C_L1 + 128 * g + 128],
                               R[:, HH * hh:HH * hh + HH]))
                return H1

            def stage_B2_half(sp, H1, ev):
                HH = W2T // 2
                H2 = actp.tile([128, 8 * W2T], FPR, tag="H2")
                H3 = actp.tile([128, 8 * W2T], FPR, tag="H3")
                layer_half(sp, ev, "l2", H2, b2v,
                           lambda g, hh: (
                               wsr[:, CW2:CW2 + 128],
                               H1[:, W2T * g + HH * hh:W2T * g + HH * hh + HH]))
                layer_half(sp, ev, "l3", H3, b3v,
                           lambda g, hh: (
                               wsr[:, CW3:CW3 + 128],
                               H2[:, W2T * g + HH * hh:W2T * g + HH * hh + HH]))
                # L4 runs full-width: it is the span tail and 256-col moving
                # operands keep the fp32r fast path
                ps4 = ps4p.tile([128, T], FP, tag="ps4")
                H3r = H3.rearrange("p (g c P j) -> p g c P j", g=8, c=NB // 2, P=2)
                k = 0
                for g in range(8):
                    for Pb in range(2):
                        nc.tensor.matmul(
                            ps4[:, :],
                            wsr[:, CL4 + 128 * (2 * g + Pb):
                                CL4 + 128 * (2 * g + Pb) + 128],
                            H3r[:, g, :, Pb, :],
                            start=(k == 0), stop=False)
                        k += 1
                nc.tensor.matmul(ps4[:, :], b4row, ones,
                                 start=False, stop=True)
                Y = iop.tile([128, T], FP, tag="Y")
                nc.vector.transpose(Y[:, :], ps4[:, :])
                nc.sync.dma_start(out_d[sp], Y[:, :])

            # Software pipeline with a 1-span skew: the next span's preamble
            # is emitted between this span's L1 evacs and L2 matmuls, so the
            # elementwise engines interleave preamble work into the gaps the
            # PE leaves while it runs L2..L4, and R(sp+1) is ready well
            # before the PE needs it. Span 0 runs in smaller chunks purely to
            # shorten the pipeline-fill ramp.
            if ramp_chunks:
                R_cur = stage_A_quartered(0)
                load_l1_weights()
                load_late_weights()
                ev = iter((evac_sched * 2)[:24])
                H1 = stage_B1_half(0, R_cur, ev)
                R_cur = stage_A(1) if nspans > 1 else None
                stage_B2_half(0, H1, ev)
                first = 1
            else:
                R_cur = stage_A(0)
                load_l1_weights()
                load_late_weights()
                first = 0
            flip = {"A": "D", "D": "A", "P": "P"}
            for sp in range(first, nspans):
                sched = evac_sched
                if span_alt and sp % 2 == 1:
                    sched = "".join(flip[c] for c in evac_sched)
                elif evac_sched2 is not None and sp % 2 == 1:
                    sched = evac_sched2
                ev = iter(sched)
                H1 = stage_B1(sp, R_cur, ev)
                if sp + 1 < nspans:
                    R_cur = stage_A(sp + 1)
                stage_B2(sp, H1, ev)

    split_sync_waits(nc)
    return nc


_CACHE = {}


def _get_program(T, nspans):
    key = (T, nspans)
    if key not in _CACHE:
        _CACHE[key] = build_program(T, nspans)
    return _CACHE[key]


def make_in_maps(F, W1, b1, W2, b2, W3, b3, W4, b4, T=T_DEF, nspans=NSPANS_DEF):
    Fr = np.ascontiguousarray(F, dtype=np.float32).reshape(-1, 4)
    ncore = 128 * T * nspans
    assert Fr.shape[0] == ncore * NCORES
    wpack, bvec, w1b = pack_weights(
        np.asarray(W1, np.float32), np.asarray(b1, np.float32),
        np.asarray(W2, np.float32), np.asarray(b2, np.float32),
        np.asarray(W3, np.float32), np.asarray(b3, np.float32),
        np.asarray(W4, np.float32), np.asarray(b4, np.float32), T)
    return [
        {"f": Fr[c * ncore:(c + 1) * ncore].reshape(nspans, 128, 4 * T),
         "wpack": wpack, "bvec": bvec, "w1b": w1b}
        for c in range(NCORES)
    ]


def kernel(F, W1, b1, W2, b2, W3, b3, W4, b4):
    """Full-input entry point: shard across 8 NeuronCores, run, gather."""
    from concourse.bass_utils import run_bass_kernel_spmd

    T, nspans = T_DEF, NSPANS_DEF
    nc = _get_program(T, nspans)
    in_maps = make_in_maps(F, W1, b1, W2, b2, W3, b3, W4, b4, T, nspans)
    res = run_bass_kernel_spmd(nc, in_maps, core_ids=list(range(NCORES)),
                               trace=False)
    out = np.concatenate(
        [res.results[c]["out"].reshape(-1) for c in range(NCORES)])
    return out.reshape(-1, 1).astype(np.float32)
